# revision 11
# baseline (speedup 1.0000x reference)
"""AttentionBlock (GroupNorm32 + 8-head global self-attention + proj + residual)
on 8 TRN2 NeuronCores, data-parallel over batch (B=8 -> 1 image per core).

Per-core layout ([C=512, N=1024] slice, channels on partitions):
  Startup: x shipped twice (bf16 for GN/QKV critical path, f32 late for the
  residual); per-ct DMAs across sync+scalar queues; weights per-kt on gpsimd.
  GroupNorm per-ct as tiles arrive (bn_stats -> per-group selector matmul ->
  expand, affine folded into A,B).  GN psum lives in psX bank 1; qk tiles
  accumulate nt-outer so their bank-1 writes are FIFO-gated behind hn3
  (PSUM bank collisions between engines are fatal and the tracker is
  address-level, so bank sharing is managed by emission order).
  Attention: S^T per (m-tile, head-parity) in double-buffered psS halves
  [128,1024]; exp split between ACT (Exp activation) and DVE (Schraudolph
  bit-trick: round(A*s+B) as int16 == bf16 exp approximation, one
  tensor_scalar).  PV per (head, nt) chain accumulates [66,512] with a
  ones-row denominator; evacuation fused with softmax normalization
  (tensor_tensor multiply by DRAM-bounce-broadcast reciprocal); reciprocal
  on repacked [128,2,4] layout.  proj: partial kt accumulation early in
  psS/psV, residual + bias via scalar_tensor_tensor, split output DMA.
"""
import math

import numpy as np

C = 512
NH = 8
D = 64
N = 1024
GROUPS = 32
GS = C // GROUPS  # 16 channels per group
EPS = 1e-5
B = 8
NT = N // 512     # 2 n-tiles of 512
CT = C // 128     # 4 channel tiles
MT = N // 128     # 8 m-tiles (sequence on partitions)

TRACE = False     # test.py flips this for profiling runs

_cache = {}


def _build(with_bias):
    import concourse.bass as bass
    import concourse.bacc as bacc
    import concourse.tile as tile
    import concourse.mybir as mybir

    F32 = mybir.dt.float32
    F32R = mybir.dt.float32r
    BF16 = mybir.dt.bfloat16
    I16 = mybir.dt.int16
    AF = mybir.ActivationFunctionType
    ALU = mybir.AluOpType
    nc = bacc.Bacc("TRN2", target_bir_lowering=False, debug=False,
                   enable_asserts=False, num_devices=1)

    x_d = nc.dram_tensor("x", [C, N], F32, kind="ExternalInput").ap()
    xbf_d = nc.dram_tensor("x_bf", [C, N], BF16, kind="ExternalInput").ap()
    qkv_wT_d = nc.dram_tensor("qkv_wT", [C, 3 * C], BF16, kind="ExternalInput").ap()
    proj_wT_d = nc.dram_tensor("proj_wT", [C, C], BF16, kind="ExternalInput").ap()
    qk_bias_d = nc.dram_tensor("qk_bias", [2 * C, 1], F32, kind="ExternalInput").ap()
    gn_w_d = nc.dram_tensor("gn_w", [C, 1], F32, kind="ExternalInput").ap()
    gn_b_d = nc.dram_tensor("gn_b", [C, 1], F32, kind="ExternalInput").ap()
    proj_be_d = nc.dram_tensor("proj_be", [C, 1], F32, kind="ExternalInput").ap()
    sel_d = nc.dram_tensor("sel", [128, 8], F32R, kind="ExternalInput").ap()
    expander_d = nc.dram_tensor("expander", [8, 128], F32R, kind="ExternalInput").ap()
    rs_dram = nc.dram_tensor("rs_scratch", [NH, N], F32, kind="Internal").ap()
    rs2_dram = nc.dram_tensor("rs2_scratch", [NH, N], F32, kind="Internal").ap()
    out_d = nc.dram_tensor("out", [C, N], F32, kind="ExternalOutput").ap()

    x_r = x_d.rearrange("(t p) n -> p t n", p=128)
    xbf_r = xbf_d.rearrange("(t p) n -> p t n", p=128)
    qkv_r = qkv_wT_d.rearrange("(t p) o -> p t o", p=128)
    proj_r = proj_wT_d.rearrange("(t p) o -> p t o", p=128)
    out_r = out_d.rearrange("(t p) n -> p t n", p=128)

    scale = float(D) ** -0.5
    # Schraudolph exp-as-bf16: bf16_bits(exp(scale*s)) ~= round(A*s + B)
    SCH_A = (2.0 ** 23) / math.log(2.0) / 65536.0 * scale
    SCH_B = (127.0 * 2 ** 23 - 368000.0) / 65536.0
    # which S^T groups (g = 2*mt + hh) run on DVE instead of ACT, pairs 1-3
    EXP_DVE = {1, 3, 5, 9, 13}

    with tile.TileContext(nc) as tc:
        with tc.tile_pool(name="const", bufs=1) as const, \
             tc.tile_pool(name="big", bufs=1) as big, \
             tc.tile_pool(name="pT_pool", bufs=4) as pT_pool, \
             tc.tile_pool(name="small", bufs=2) as small, \
             tc.tile_pool(name="norm", bufs=4) as norm, \
             tc.tile_pool(name="psS_p", bufs=1, space="PSUM") as psS_p, \
             tc.tile_pool(name="psV_p", bufs=1, space="PSUM") as psV_p, \
             tc.tile_pool(name="psX_p", bufs=1, space="PSUM") as psX_p:

            # ---- PSUM: 3 fixed tiles, manually sliced (8 banks total) ----
            psS = psS_p.tile([128, 2048], F32)   # 4 banks: S^T halves, q0/k0, proj01
            psV = psV_p.tile([128, 1024], F32)   # 2 banks: v staging, PV 0,1, proj3
            psX = psX_p.tile([128, 1024], F32)   # 2 banks: GN(bank1), qk, PV 2,3, proj2

            # ---- constants / weights (gpsimd queue) ----
            sel = const.tile([128, 8], F32R)
            expander = const.tile([8, 128], F32R)
            gn_w = const.tile([128, CT, 1], F32)
            gn_b = const.tile([128, CT, 1], F32)
            proj_be = const.tile([128, CT, 1], F32)
            qkv_wT = const.tile([128, CT, 3 * C], BF16)
            proj_wT = const.tile([128, CT, C], BF16)
            eps_t = const.tile([8, 1], F32)
            qk_bias = const.tile([128, 2 * CT, 1], F32)

            nc.gpsimd.dma_start(out=sel, in_=sel_d)
            nc.gpsimd.dma_start(out=expander, in_=expander_d)
            nc.gpsimd.dma_start(out=gn_w, in_=gn_w_d.rearrange("(t p) o -> p t o", p=128))
            nc.gpsimd.dma_start(out=gn_b, in_=gn_b_d.rearrange("(t p) o -> p t o", p=128))
            nc.gpsimd.dma_start(out=proj_be, in_=proj_be_d.rearrange("(t p) o -> p t o", p=128))
            if with_bias:
                nc.gpsimd.dma_start(out=qk_bias,
                                    in_=qk_bias_d.rearrange("(t p) o -> p t o", p=128))
            for kt in range(CT):
                nc.gpsimd.dma_start(out=qkv_wT[:, kt, :], in_=qkv_r[:, kt, :])
            nc.gpsimd.dma_start(out=proj_wT, in_=proj_r)
            nc.vector.memset(eps_t, EPS)

            # ---- input x (bf16, critical path): 4 per-ct DMAs, 2 queues ----
            xb_sb = big.tile([128, CT, N], BF16)
            for ci in range(CT):
                q = nc.sync if ci % 2 == 0 else nc.scalar
                q.dma_start(out=xb_sb[:, ci, :], in_=xbf_r[:, ci, :])
            # f32 x for the residual, loaded behind the weights (gpsimd)
            x_sb = big.tile([128, CT, N], F32)
            nc.gpsimd.dma_start(out=x_sb, in_=x_r)

            # ---- GroupNorm, per-ct pipeline (psum in psX bank 1) ----
            hn = big.tile([128, CT, N], BF16)
            for ci in range(CT):
                bstats = norm.tile([128, 2, 6], F32, tag="bst")
                xv = xb_sb[:, ci, :].rearrange("p (s n) -> p s n", s=2)
                for s in range(2):
                    nc.vector.bn_stats(out=bstats[:, s, :], in_=xv[:, s, :])
                mv = norm.tile([128, 2], F32, tag="mv")
                nc.vector.bn_aggr(out=mv, in_=bstats)
                # srhs: col0 = mean_c, col1 = var_c + mean_c^2 (= E[x^2])
                srhs = norm.tile([128, 2], F32R, tag="srhs")
                nc.vector.tensor_copy(out=srhs[:, 0:1], in_=mv[:, 0:1])
                nc.vector.tensor_tensor(out=srhs[:, 1:2], in0=mv[:, 0:1],
                                        in1=mv[:, 0:1], op=ALU.mult)
                nc.vector.tensor_tensor(out=srhs[:, 1:2], in0=srhs[:, 1:2],
                                        in1=mv[:, 1:2], op=ALU.add)
                gp = psX[0:8, 512 + 16 * ci:512 + 16 * ci + 2]
                nc.tensor.matmul(gp, sel[:], srhs[:], start=True, stop=True)
                gms = norm.tile([8, 2], F32, tag="gms")
                nc.vector.tensor_copy(out=gms, in_=gp)
                gvar = norm.tile([8, 1], F32, tag="gvar")
                grp2 = norm.tile([8, 2], F32R, tag="grp2")
                nc.vector.tensor_tensor(out=gvar, in0=gms[:, 0:1], in1=gms[:, 0:1],
                                        op=ALU.mult)
                nc.vector.tensor_tensor(out=gvar, in0=gms[:, 1:2], in1=gvar,
                                        op=ALU.subtract)
                nc.scalar.activation(out=gvar, in_=gvar, func=AF.Sqrt, bias=eps_t,
                                     scale=1.0)
                nc.vector.reciprocal(out=gvar, in_=gvar)
                nc.vector.tensor_copy(out=grp2[:, 0:1], in_=gms[:, 0:1])
                nc.vector.tensor_copy(out=grp2[:, 1:2], in_=gvar)
                ep = psX[:, 576 + 16 * ci:576 + 16 * ci + 2]
                nc.tensor.matmul(ep, expander[:], grp2[:], start=True, stop=True)
                A = norm.tile([128, 1], F32, tag="A")
                Bb = norm.tile([128, 1], F32, tag="Bb")
                nc.vector.tensor_tensor(out=A, in0=ep[:, 1:2], in1=gn_w[:, ci, :],
                                        op=ALU.mult)
                nc.vector.tensor_tensor(out=Bb, in0=ep[:, 0:1], in1=A, op=ALU.mult)
                nc.vector.tensor_tensor(out=Bb, in0=gn_b[:, ci, :], in1=Bb,
                                        op=ALU.subtract)
                nc.vector.tensor_scalar(out=hn[:, ci, :], in0=xb_sb[:, ci, :],
                                        scalar1=A, scalar2=Bb,
                                        op0=ALU.mult, op1=ALU.add)

            # ---- data tiles for attention ----
            q_sb = big.tile([128, CT, N], BF16)
            k_sb = big.tile([128, CT, N], BF16)
            vT = big.tile([128, MT, NH, D + 2], BF16)
            oT = big.tile([128, CT, N], BF16)
            out_sb = big.tile([128, CT, N], F32)
            nc.vector.memset(vT[:, :, :, D:D + 1], 1.0)
            nc.vector.memset(vT[:, :, :, D + 1:D + 2], 0.0)

            # ---- emission helpers ----
            def qk_tile(i, on_act, base, off):
                """QKV tile i (0-3 = q ct, 4-7 = k ct). nt-outer: the nt1
                chain's first MM is FIFO-gated behind nt0's kt3 (needs hn3),
                so bank off+512 is only written once GN is fully done."""
                dest = q_sb if i < CT else k_sb
                ci = i % CT
                for nt in range(NT):
                    for kt in range(CT):
                        nc.tensor.matmul(
                            base[:, off + 512 * nt: off + 512 * (nt + 1)],
                            qkv_wT[:, kt, 128 * i:128 * (i + 1)],
                            hn[:, kt, 512 * nt:512 * (nt + 1)],
                            start=(kt == 0), stop=(kt == CT - 1))
                if with_bias:
                    nc.vector.tensor_scalar(out=dest[:, ci, :],
                                            in0=base[:, off:off + 1024],
                                            scalar1=qk_bias[:, i, :], scalar2=None,
                                            op0=ALU.add)
                elif on_act:
                    nc.scalar.activation(out=dest[:, ci, :],
                                         in_=base[:, off:off + 1024], func=AF.Copy)
                else:
                    nc.vector.tensor_copy(out=dest[:, ci, :],
                                          in_=base[:, off:off + 1024])

            def v_tile(mt):
                """v for n-block mt into psV half, evac to vT (head-interleaved)."""
                half = (mt % 2) * 512
                for kt in range(CT):
                    nc.tensor.matmul(psV[:, half:half + 512],
                                     hn[:, kt, 128 * mt:128 * (mt + 1)],
                                     qkv_wT[:, kt, 2 * C:3 * C],
                                     start=(kt == 0), stop=(kt == CT - 1))
                nc.vector.tensor_copy(
                    out=vT[:, mt, :, 0:D],
                    in_=psV[:, half:half + 512].rearrange("p (h d) -> p h d", h=NH))

            pT_tiles = {}

            def st_half(t, g, on_dve):
                """S^T for head pair t, group g = 2*mt + hh, into psS half
                g%2 (double-buffered); exp on ACT or DVE-Schraudolph."""
                mt, hh = g // 2, g % 2
                qp = hh * 64
                half = psS[:, 1024 * (g % 2):1024 * (g % 2) + 1024]
                for nt in range(NT):
                    nc.tensor.matmul(
                        half[:, 512 * nt:512 * (nt + 1)],
                        k_sb[qp:qp + 64, t, 128 * mt:128 * (mt + 1)],
                        q_sb[qp:qp + 64, t, 512 * nt:512 * (nt + 1)],
                        start=True, stop=True)
                pTt = pT_tiles[t]
                if on_dve:
                    nc.vector.tensor_scalar(
                        out=pTt.bitcast(I16)[:, hh, mt, :], in0=half,
                        scalar1=SCH_A, scalar2=SCH_B,
                        op0=ALU.mult, op1=ALU.add)
                else:
                    nc.scalar.activation(out=pTt[:, hh, mt, :], in_=half,
                                         func=AF.Exp, scale=scale)

            # PV chain psum slots: chain 0,1 -> psV halves; 2,3 -> psX halves
            def pv_slot(chain):
                base = psV if chain < 2 else psX
                off = 512 * (chain % 2)
                return base[0:D + 2, off:off + 512]

            def pv_chunk(t, chain, part):
                """4 MMs of PV chain (0=h_ev/nt0, 1=h_od/nt0, 2=h_ev/nt1,
                3=h_od/nt1), part 0/1 = m-tiles 0-3 / 4-7."""
                hh = chain % 2
                nt = chain // 2
                h = 2 * t + hh
                slot = pv_slot(chain)
                pTt = pT_tiles[t]
                for mt in range(4 * part, 4 * part + 4):
                    nc.tensor.matmul(slot,
                                     vT[:, mt, h, :],
                                     pTt[:, hh, mt, 512 * nt:512 * (nt + 1)],
                                     start=(mt == 0), stop=(mt == MT - 1))
                if part == 1:
                    rs = small.tile([1, 512], F32, tag="rs", bufs=4,
                                    name=f"rs_{h}_{nt}")
                    nc.vector.tensor_copy(out=rs, in_=slot[D:D + 1, :])
                    nc.sync.dma_start(out=rs_dram[h:h + 1, 512 * nt:512 * (nt + 1)],
                                      in_=rs)

            def recip_batch(t, nt):
                """1/denominator for heads 2t,2t+1, n-half nt, repacked [128,2,4]."""
                rg = small.tile([128, 2, 4], F32, tag="rg", bufs=2,
                                name=f"rg_{t}_{nt}")
                src = rs_dram[2 * t:2 * t + 2, 512 * nt:512 * (nt + 1)]
                nc.sync.dma_start(out=rg,
                                  in_=src.rearrange("h (p f) -> p h f", p=128))
                nc.vector.reciprocal(out=rg, in_=rg)
                nc.sync.dma_start(
                    out=rs2_dram[2 * t:2 * t + 2, 512 * nt:512 * (nt + 1)].rearrange(
                        "h (p f) -> p h f", p=128),
                    in_=rg)

            bc_tiles = {}

            def bc_load(t, chain):
                """Broadcast 1/denom to 64 partitions via DRAM-bounce DMA."""
                hh, nt = chain % 2, chain // 2
                h = 2 * t + hh
                qp = hh * 64
                key = (t, nt)
                if key not in bc_tiles:
                    bc_tiles[key] = small.tile([128, 512], F32, tag=f"bc{nt}",
                                               bufs=2, name=f"bc_{t}_{nt}")
                bc = bc_tiles[key]
                srcap = rs2_dram[h:h + 1, 512 * nt:512 * (nt + 1)]
                nc.gpsimd.dma_start(out=bc[qp:qp + 64, :],
                                    in_=bass.AP(tensor=srcap.tensor,
                                                offset=srcap.offset,
                                                ap=[[0, 64]] + list(srcap.ap[1:])))

            def pv_evac(t, chain):
                """Fused evacuate+normalize: oT = psum_o * (1/denom)."""
                hh, nt = chain % 2, chain // 2
                qp = hh * 64
                slot = pv_slot(chain)
                bc = bc_tiles[(t, nt)]
                nc.vector.tensor_tensor(
                    out=oT[qp:qp + 64, t, 512 * nt:512 * (nt + 1)],
                    in0=slot[0:D, :], in1=bc[qp:qp + 64, :], op=ALU.mult)

            def proj_chunk(ot, kts, base, off, first, last):
                for kt in kts:
                    for nt in range(NT):
                        nc.tensor.matmul(
                            base[:, off + 512 * nt:off + 512 * (nt + 1)],
                            proj_wT[:, kt, 128 * ot:128 * (ot + 1)],
                            oT[:, kt, 512 * nt:512 * (nt + 1)],
                            start=(kt == kts[0] and first),
                            stop=(kt == kts[-1] and last))

            def proj_finish(ot, base, off):
                nc.vector.scalar_tensor_tensor(
                    out=out_sb[:, ot, :], in0=base[:, off:off + 1024],
                    scalar=proj_be[:, ot, :], in1=x_sb[:, ot, :],
                    op0=ALU.add, op1=ALU.add)
                q = nc.sync if ot % 2 == 0 else nc.scalar
                q.dma_start(out=out_r[:, ot, :], in_=out_sb[:, ot, :])

            def stage_b16(pv, g):
                """Denominator chain steps for pair pv, at slot g (0-15) of
                the following loop."""
                if g == 7:
                    recip_batch(pv, 0)
                    bc_load(pv, 0)
                    bc_load(pv, 1)
                elif g == 10:
                    pv_evac(pv, 0)
                elif g == 11:
                    pv_evac(pv, 1)
                elif g == 15:
                    recip_batch(pv, 1)
                    bc_load(pv, 2)
                    bc_load(pv, 3)

            def alloc_pT(t):
                pT_tiles[t] = pT_pool.tile([128, 2, MT, N], BF16, tag="pT", bufs=2,
                                           name=f"pT_{t}")

            # ---- pipeline emission ----
            # q0, k0 first (psS halves) so pair-0 S^T can start early
            qk_tile(0, on_act=True, base=psS, off=0)
            qk_tile(4, on_act=True, base=psS, off=1024)

            # pair 0: S^T+exp (all ACT; PE-bound anyway) + v tiles (psV) +
            # remaining qk tiles (psX)
            alloc_pT(0)
            rest = [1, 5, 2, 6, 3, 7]        # q1,k1,q2,k2,q3,k3
            for g in range(16):
                if g % 2 == 0:
                    v_tile(g // 2)
                elif g < 13:
                    qk_tile(rest[g // 2], on_act=False, base=psX, off=0)
                st_half(0, g, on_dve=False)

            # pairs 1..3: PV(t-1) + S^T(t) + exp (ACT/DVE split) + stage_b
            for t in range(1, 4):
                alloc_pT(t)
                pv = t - 1
                for g in range(16):
                    if g == 0 and pv >= 1:
                        pv_evac(pv - 1, 2)
                    if g == 1 and pv >= 1:
                        pv_evac(pv - 1, 3)
                    if g % 2 == 0:
                        pv_chunk(pv, g // 4, (g // 2) % 2)
                    st_half(t, g, on_dve=(g in EXP_DVE))
                    stage_b16(pv, g)
                del pT_tiles[t - 1]

            # PV(3) + early proj partials (ot0/ot1 in psS halves, ot3 in psV)
            proj_sched = {1: (0, 0), 3: (1, 0), 5: (0, 1), 7: (1, 1),
                          9: (0, 2), 11: (1, 2)}
            for g in range(16):
                if g == 0:
                    pv_evac(2, 2)
                if g == 1:
                    pv_evac(2, 3)
                if g % 2 == 0:
                    pv_chunk(3, g // 4, (g // 2) % 2)
                ps = proj_sched.get(g)
                if ps is not None:
                    ot, kt = ps
                    proj_chunk(ot, [kt], psS, 1024 * ot, first=(kt == 0),
                               last=False)
                if g == 12:
                    proj_chunk(3, [0], psV, 0, first=True, last=False)
                if g == 13:
                    proj_chunk(3, [1], psV, 0, first=False, last=False)
                if g == 14:
                    proj_chunk(3, [2], psV, 0, first=False, last=False)
                stage_b16(3, g)
            pv_evac(3, 2)
            pv_evac(3, 3)

            # ---- finish projection + residual ----
            proj_chunk(0, [3], psS, 0, first=False, last=True)
            proj_finish(0, psS, 0)
            proj_chunk(1, [3], psS, 1024, first=False, last=True)
            proj_finish(1, psS, 1024)
            proj_chunk(3, [3], psV, 0, first=False, last=True)
            proj_finish(3, psV, 0)
            proj_chunk(2, [0, 1, 2, 3], psX, 0, first=True, last=True)
            proj_finish(2, psX, 0)

    nc.compile()
    return nc


def _host_prep(x, gn_w, gn_b, qkv_w, qkv_b, proj_w, proj_b):
    xf = np.ascontiguousarray(x.reshape(B, C, N), dtype=np.float32)
    import ml_dtypes
    qkv_wT = np.ascontiguousarray(qkv_w.T).astype(ml_dtypes.bfloat16)
    proj_wT = np.ascontiguousarray(proj_w.T).astype(ml_dtypes.bfloat16)
    proj_be = (proj_b + proj_w @ qkv_b[2 * C:]).astype(np.float32).reshape(C, 1)
    qk_bias = np.ascontiguousarray(qkv_b[:2 * C], dtype=np.float32).reshape(2 * C, 1)
    cid = np.arange(128)
    sel = ((cid[:, None] // GS == np.arange(8)[None, :]) / GS).astype(np.float32)
    expander = np.ascontiguousarray(
        (cid[:, None] // GS == np.arange(8)[None, :]).T.astype(np.float32))
    shared = {
        "qkv_wT": qkv_wT, "proj_wT": proj_wT, "qk_bias": qk_bias,
        "gn_w": np.asarray(gn_w, np.float32).reshape(C, 1),
        "gn_b": np.asarray(gn_b, np.float32).reshape(C, 1),
        "proj_be": proj_be, "sel": sel, "expander": expander,
    }
    return [{**shared, "x": np.ascontiguousarray(xf[i]),
             "x_bf": xf[i].astype(ml_dtypes.bfloat16)} for i in range(B)]


def kernel(x, gn_w, gn_b, qkv_w, qkv_b, proj_w, proj_b):
    from concourse import bass_utils
    in_maps = _host_prep(np.asarray(x), np.asarray(gn_w), np.asarray(gn_b),
                         np.asarray(qkv_w), np.asarray(qkv_b),
                         np.asarray(proj_w), np.asarray(proj_b))
    with_bias = bool(np.any(np.asarray(qkv_b)[:2 * C] != 0.0))
    key = ("nc", with_bias)
    if key not in _cache:
        _cache[key] = _build(with_bias)
    res = bass_utils.run_bass_kernel_spmd(_cache[key], in_maps,
                                          core_ids=list(range(B)), trace=TRACE)
    _cache["last_result"] = res
    out = np.stack([res.results[i]["out"] for i in range(B)])
    return out.reshape(B, C, 32, 32).astype(np.float32)


# revision 13
# speedup vs baseline: 1.3901x; 1.3901x over previous
"""AttentionBlock (GroupNorm32 + 8-head global self-attention + proj + residual)
on 8 TRN2 NeuronCores, data-parallel over batch (B=8 -> 1 image per core).

Per-core layout ([C=512, N=1024] slice, channels on partitions):
  Startup: x shipped twice (bf16 for GN/QKV critical path, f32 late for the
  residual); per-ct DMAs across sync+scalar queues; weights per-kt on gpsimd.
  GroupNorm per-ct as tiles arrive (bn_stats -> per-group selector matmul ->
  expand, affine folded into A,B).  GN psum lives in psX bank 1; qk tiles
  accumulate nt-outer so their bank-1 writes are FIFO-gated behind hn3
  (PSUM bank collisions between engines are fatal and the tracker is
  address-level, so bank sharing is managed by emission order).
  Attention: S^T per (m-tile, head-parity) in double-buffered psS halves
  [128,1024]; exp split between ACT (Exp activation) and DVE (Schraudolph
  bit-trick: round(A*s+B) as int16 == bf16 exp approximation, one
  tensor_scalar).  PV per (head, nt) chain accumulates [66,512] with a
  ones-row denominator; evacuation fused with softmax normalization
  (tensor_tensor multiply by DRAM-bounce-broadcast reciprocal); reciprocal
  on repacked [128,2,4] layout.  proj: partial kt accumulation early in
  psS/psV, residual + bias via scalar_tensor_tensor, split output DMA.
"""
import math

import numpy as np

C = 512
NH = 8
D = 64
N = 1024
GROUPS = 32
GS = C // GROUPS  # 16 channels per group
EPS = 1e-5
B = 8
NT = N // 512     # 2 n-tiles of 512
CT = C // 128     # 4 channel tiles
MT = N // 128     # 8 m-tiles (sequence on partitions)

TRACE = False     # test.py flips this for profiling runs

_cache = {}


def _build(with_bias):
    import concourse.bass as bass
    import concourse.bacc as bacc
    import concourse.tile as tile
    import concourse.mybir as mybir

    F32 = mybir.dt.float32
    F32R = mybir.dt.float32r
    BF16 = mybir.dt.bfloat16
    I16 = mybir.dt.int16
    AF = mybir.ActivationFunctionType
    ALU = mybir.AluOpType
    nc = bacc.Bacc("TRN2", target_bir_lowering=False, debug=False,
                   enable_asserts=False, num_devices=1)

    x_d = nc.dram_tensor("x", [C, N], F32, kind="ExternalInput").ap()
    xbf_d = nc.dram_tensor("x_bf", [C, N], BF16, kind="ExternalInput").ap()
    qkv_wT_d = nc.dram_tensor("qkv_wT", [C, 3 * C], BF16, kind="ExternalInput").ap()
    proj_wT_d = nc.dram_tensor("proj_wT", [C, C], BF16, kind="ExternalInput").ap()
    qk_bias_d = nc.dram_tensor("qk_bias", [2 * C, 1], F32, kind="ExternalInput").ap()
    gn_w_d = nc.dram_tensor("gn_w", [C, 1], F32, kind="ExternalInput").ap()
    gn_b_d = nc.dram_tensor("gn_b", [C, 1], F32, kind="ExternalInput").ap()
    proj_be_d = nc.dram_tensor("proj_be", [C, 1], F32, kind="ExternalInput").ap()
    sel_d = nc.dram_tensor("sel", [128, 8], F32R, kind="ExternalInput").ap()
    expander_d = nc.dram_tensor("expander", [8, 128], F32R, kind="ExternalInput").ap()
    rs_dram = nc.dram_tensor("rs_scratch", [NH, N], F32, kind="Internal").ap()
    rs2_dram = nc.dram_tensor("rs2_scratch", [NH, N], F32, kind="Internal").ap()
    out_d = nc.dram_tensor("out", [C, N], F32, kind="ExternalOutput").ap()

    x_r = x_d.rearrange("(t p) n -> p t n", p=128)
    xbf_r = xbf_d.rearrange("(t p) n -> p t n", p=128)
    qkv_r = qkv_wT_d.rearrange("(t p) o -> p t o", p=128)
    proj_r = proj_wT_d.rearrange("(t p) o -> p t o", p=128)
    out_r = out_d.rearrange("(t p) n -> p t n", p=128)

    scale = float(D) ** -0.5
    # Schraudolph exp-as-bf16: bf16_bits(exp(scale*s)) ~= round(A*s + B)
    SCH_A = (2.0 ** 23) / math.log(2.0) / 65536.0 * scale
    SCH_B = (127.0 * 2 ** 23 - 368000.0) / 65536.0
    # which S^T groups (g = 2*mt + hh) run on DVE instead of ACT, pairs 1-3
    EXP_DVE = {1, 3, 5, 9, 13}

    with tile.TileContext(nc) as tc:
        with tc.tile_pool(name="const", bufs=1) as const, \
             tc.tile_pool(name="big", bufs=1) as big, \
             tc.tile_pool(name="pT_pool", bufs=4) as pT_pool, \
             tc.tile_pool(name="small", bufs=2) as small, \
             tc.tile_pool(name="norm", bufs=4) as norm, \
             tc.tile_pool(name="psSa_p", bufs=1, space="PSUM") as psSa_p, \
             tc.tile_pool(name="psSb_p", bufs=1, space="PSUM") as psSb_p, \
             tc.tile_pool(name="psVa_p", bufs=1, space="PSUM") as psVa_p, \
             tc.tile_pool(name="psVb_p", bufs=1, space="PSUM") as psVb_p, \
             tc.tile_pool(name="psXa_p", bufs=1, space="PSUM") as psXa_p, \
             tc.tile_pool(name="psXb_p", bufs=1, space="PSUM") as psXb_p:

            # ---- PSUM: 6 fixed tiles (8 banks). Separate tiles because the
            # dependency tracker serializes at tile granularity: the S^T/exp
            # double-buffer only overlaps if the halves are distinct tiles.
            psSa = psSa_p.tile([128, 1024], F32)  # S^T even groups, q0, proj 0/2
            psSb = psSb_p.tile([128, 1024], F32)  # S^T odd groups, k0, proj 1/3
            psVa = psVa_p.tile([128, 512], F32)   # v even, PV chain 0
            psVb = psVb_p.tile([128, 512], F32)   # v odd, PV chain 1
            psXa = psXa_p.tile([128, 512], F32)   # qk nt0, PV chain 2
            psXb = psXb_p.tile([128, 512], F32)   # GN, qk nt1, PV chain 3

            # ---- constants / weights (gpsimd queue) ----
            sel = const.tile([128, 8], F32R)
            expander = const.tile([8, 128], F32R)
            gn_w = const.tile([128, CT, 1], F32)
            gn_b = const.tile([128, CT, 1], F32)
            proj_be = const.tile([128, CT, 1], F32)
            qkv_wT = const.tile([128, CT, 3 * C], BF16)
            proj_wT = const.tile([128, CT, C], BF16)
            eps_t = const.tile([8, 1], F32)
            qk_bias = const.tile([128, 2 * CT, 1], F32)

            nc.gpsimd.dma_start(out=sel, in_=sel_d)
            nc.gpsimd.dma_start(out=expander, in_=expander_d)
            nc.gpsimd.dma_start(out=gn_w, in_=gn_w_d.rearrange("(t p) o -> p t o", p=128))
            nc.gpsimd.dma_start(out=gn_b, in_=gn_b_d.rearrange("(t p) o -> p t o", p=128))
            nc.gpsimd.dma_start(out=proj_be, in_=proj_be_d.rearrange("(t p) o -> p t o", p=128))
            if with_bias:
                nc.gpsimd.dma_start(out=qk_bias,
                                    in_=qk_bias_d.rearrange("(t p) o -> p t o", p=128))
            for kt in range(CT):
                nc.gpsimd.dma_start(out=qkv_wT[:, kt, :], in_=qkv_r[:, kt, :])
            nc.gpsimd.dma_start(out=proj_wT, in_=proj_r)
            nc.vector.memset(eps_t, EPS)

            # ---- input x (bf16, critical path): 4 per-ct DMAs, 2 queues ----
            xb_sb = big.tile([128, CT, N], BF16)
            for ci in range(CT):
                q = nc.sync if ci % 2 == 0 else nc.scalar
                q.dma_start(out=xb_sb[:, ci, :], in_=xbf_r[:, ci, :])
            # f32 x for the residual, loaded behind the weights (gpsimd)
            x_sb = big.tile([128, CT, N], F32)
            nc.gpsimd.dma_start(out=x_sb, in_=x_r)

            # ---- GroupNorm, per-ct pipeline (psum in psX bank 1) ----
            hn = big.tile([128, CT, N], BF16)
            for ci in range(CT):
                bstats = norm.tile([128, 2, 6], F32, tag="bst")
                xv = xb_sb[:, ci, :].rearrange("p (s n) -> p s n", s=2)
                for s in range(2):
                    nc.vector.bn_stats(out=bstats[:, s, :], in_=xv[:, s, :])
                mv = norm.tile([128, 2], F32, tag="mv")
                nc.vector.bn_aggr(out=mv, in_=bstats)
                # srhs: col0 = mean_c, col1 = var_c + mean_c^2 (= E[x^2])
                srhs = norm.tile([128, 2], F32R, tag="srhs")
                nc.vector.tensor_copy(out=srhs[:, 0:1], in_=mv[:, 0:1])
                nc.vector.tensor_tensor(out=srhs[:, 1:2], in0=mv[:, 0:1],
                                        in1=mv[:, 0:1], op=ALU.mult)
                nc.vector.tensor_tensor(out=srhs[:, 1:2], in0=srhs[:, 1:2],
                                        in1=mv[:, 1:2], op=ALU.add)
                gp = psXb[0:8, 16 * ci:16 * ci + 2]
                nc.tensor.matmul(gp, sel[:], srhs[:], start=True, stop=True)
                gms = norm.tile([8, 2], F32, tag="gms")
                nc.vector.tensor_copy(out=gms, in_=gp)
                gvar = norm.tile([8, 1], F32, tag="gvar")
                grp2 = norm.tile([8, 2], F32R, tag="grp2")
                nc.vector.tensor_tensor(out=gvar, in0=gms[:, 0:1], in1=gms[:, 0:1],
                                        op=ALU.mult)
                nc.vector.tensor_tensor(out=gvar, in0=gms[:, 1:2], in1=gvar,
                                        op=ALU.subtract)
                nc.scalar.activation(out=gvar, in_=gvar, func=AF.Sqrt, bias=eps_t,
                                     scale=1.0)
                nc.vector.reciprocal(out=gvar, in_=gvar)
                nc.vector.tensor_copy(out=grp2[:, 0:1], in_=gms[:, 0:1])
                nc.vector.tensor_copy(out=grp2[:, 1:2], in_=gvar)
                ep = psXb[:, 64 + 16 * ci:64 + 16 * ci + 2]
                nc.tensor.matmul(ep, expander[:], grp2[:], start=True, stop=True)
                A = norm.tile([128, 1], F32, tag="A")
                Bb = norm.tile([128, 1], F32, tag="Bb")
                nc.vector.tensor_tensor(out=A, in0=ep[:, 1:2], in1=gn_w[:, ci, :],
                                        op=ALU.mult)
                nc.vector.tensor_tensor(out=Bb, in0=ep[:, 0:1], in1=A, op=ALU.mult)
                nc.vector.tensor_tensor(out=Bb, in0=gn_b[:, ci, :], in1=Bb,
                                        op=ALU.subtract)
                nc.vector.tensor_scalar(out=hn[:, ci, :], in0=xb_sb[:, ci, :],
                                        scalar1=A, scalar2=Bb,
                                        op0=ALU.mult, op1=ALU.add)

            # ---- data tiles for attention ----
            q_sb = big.tile([128, CT, N], BF16)
            k_sb = big.tile([128, CT, N], BF16)
            vT = big.tile([128, MT, NH, D + 2], BF16)
            oT = big.tile([128, CT, N], BF16)
            out_sb = big.tile([128, CT, N], F32)
            nc.vector.memset(vT[:, :, :, D:D + 1], 1.0)
            nc.vector.memset(vT[:, :, :, D + 1:D + 2], 0.0)

            # ---- emission helpers ----
            def qk_tile(i, on_act, bases):
                """QKV tile i (0-3 = q ct, 4-7 = k ct). nt-outer: the nt1
                chain's first MM is FIFO-gated behind nt0's kt3 (needs hn3),
                so psXb is only written once GN is fully done."""
                dest = q_sb if i < CT else k_sb
                ci = i % CT
                for nt in range(NT):
                    for kt in range(CT):
                        nc.tensor.matmul(
                            bases[nt][:, 0:512] if len(bases) == 2
                            else bases[0][:, 512 * nt:512 * (nt + 1)],
                            qkv_wT[:, kt, 128 * i:128 * (i + 1)],
                            hn[:, kt, 512 * nt:512 * (nt + 1)],
                            start=(kt == 0), stop=(kt == CT - 1))
                for nt in range(NT):
                    ps = (bases[nt][:, 0:512] if len(bases) == 2
                          else bases[0][:, 512 * nt:512 * (nt + 1)])
                    dslc = dest[:, ci, 512 * nt:512 * (nt + 1)]
                    if with_bias:
                        nc.vector.tensor_scalar(out=dslc, in0=ps,
                                                scalar1=qk_bias[:, i, :],
                                                scalar2=None, op0=ALU.add)
                    elif on_act:
                        nc.scalar.activation(out=dslc, in_=ps, func=AF.Copy)
                    else:
                        nc.vector.tensor_copy(out=dslc, in_=ps)

            def v_tile(mt):
                """v for n-block mt, evac to vT (head-interleaved)."""
                base = psVa if mt % 2 == 0 else psVb
                for kt in range(CT):
                    nc.tensor.matmul(base[:, 0:512],
                                     hn[:, kt, 128 * mt:128 * (mt + 1)],
                                     qkv_wT[:, kt, 2 * C:3 * C],
                                     start=(kt == 0), stop=(kt == CT - 1))
                nc.vector.tensor_copy(
                    out=vT[:, mt, :, 0:D],
                    in_=base[:, 0:512].rearrange("p (h d) -> p h d", h=NH))

            pT_tiles = {}

            def st_half(t, g, on_dve):
                """S^T for head pair t, group g = 2*mt + hh, into psS half
                g%2 (double-buffered); exp on ACT or DVE-Schraudolph."""
                mt, hh = g // 2, g % 2
                qp = hh * 64
                half = psSa if g % 2 == 0 else psSb
                for nt in range(NT):
                    nc.tensor.matmul(
                        half[:, 512 * nt:512 * (nt + 1)],
                        k_sb[qp:qp + 64, t, 128 * mt:128 * (mt + 1)],
                        q_sb[qp:qp + 64, t, 512 * nt:512 * (nt + 1)],
                        start=True, stop=True)
                pTt = pT_tiles[t]
                if on_dve:
                    nc.vector.tensor_scalar(
                        out=pTt.bitcast(I16)[:, hh, mt, :], in0=half,
                        scalar1=SCH_A, scalar2=SCH_B,
                        op0=ALU.mult, op1=ALU.add)
                else:
                    nc.scalar.activation(out=pTt[:, hh, mt, :], in_=half,
                                         func=AF.Exp, scale=scale)

            # PV chain psum slots: one single-bank tile per chain
            def pv_slot(chain):
                return [psVa, psVb, psXa, psXb][chain][0:D + 2, 0:512]

            def pv_chunk(t, chain, part):
                """4 MMs of PV chain (0=h_ev/nt0, 1=h_od/nt0, 2=h_ev/nt1,
                3=h_od/nt1), part 0/1 = m-tiles 0-3 / 4-7."""
                hh = chain % 2
                nt = chain // 2
                h = 2 * t + hh
                slot = pv_slot(chain)
                pTt = pT_tiles[t]
                for mt in range(4 * part, 4 * part + 4):
                    nc.tensor.matmul(slot,
                                     vT[:, mt, h, :],
                                     pTt[:, hh, mt, 512 * nt:512 * (nt + 1)],
                                     start=(mt == 0), stop=(mt == MT - 1))
                if part == 1:
                    rs = small.tile([1, 512], F32, tag="rs", bufs=4,
                                    name=f"rs_{h}_{nt}")
                    nc.vector.tensor_copy(out=rs, in_=slot[D:D + 1, :])
                    nc.sync.dma_start(out=rs_dram[h:h + 1, 512 * nt:512 * (nt + 1)],
                                      in_=rs)

            def recip_batch(t, nt):
                """1/denominator for heads 2t,2t+1, n-half nt, repacked [128,2,4]."""
                rg = small.tile([128, 2, 4], F32, tag="rg", bufs=2,
                                name=f"rg_{t}_{nt}")
                src = rs_dram[2 * t:2 * t + 2, 512 * nt:512 * (nt + 1)]
                nc.sync.dma_start(out=rg,
                                  in_=src.rearrange("h (p f) -> p h f", p=128))
                nc.vector.reciprocal(out=rg, in_=rg)
                nc.sync.dma_start(
                    out=rs2_dram[2 * t:2 * t + 2, 512 * nt:512 * (nt + 1)].rearrange(
                        "h (p f) -> p h f", p=128),
                    in_=rg)

            bc_tiles = {}

            def bc_load(t, chain):
                """Broadcast 1/denom to 64 partitions via DRAM-bounce DMA."""
                hh, nt = chain % 2, chain // 2
                h = 2 * t + hh
                qp = hh * 64
                key = (t, nt)
                if key not in bc_tiles:
                    bc_tiles[key] = small.tile([128, 512], F32, tag=f"bc{nt}",
                                               bufs=2, name=f"bc_{t}_{nt}")
                bc = bc_tiles[key]
                srcap = rs2_dram[h:h + 1, 512 * nt:512 * (nt + 1)]
                nc.gpsimd.dma_start(out=bc[qp:qp + 64, :],
                                    in_=bass.AP(tensor=srcap.tensor,
                                                offset=srcap.offset,
                                                ap=[[0, 64]] + list(srcap.ap[1:])))

            def pv_evac(t, chain):
                """Fused evacuate+normalize: oT = psum_o * (1/denom)."""
                hh, nt = chain % 2, chain // 2
                qp = hh * 64
                slot = pv_slot(chain)
                bc = bc_tiles[(t, nt)]
                nc.vector.tensor_tensor(
                    out=oT[qp:qp + 64, t, 512 * nt:512 * (nt + 1)],
                    in0=slot[0:D, :], in1=bc[qp:qp + 64, :], op=ALU.mult)

            def proj_chunk(ot, kts, base, first, last):
                for kt in kts:
                    for nt in range(NT):
                        nc.tensor.matmul(
                            base[:, 512 * nt:512 * (nt + 1)],
                            proj_wT[:, kt, 128 * ot:128 * (ot + 1)],
                            oT[:, kt, 512 * nt:512 * (nt + 1)],
                            start=(kt == kts[0] and first),
                            stop=(kt == kts[-1] and last))

            def proj_finish(ot, base):
                nc.vector.scalar_tensor_tensor(
                    out=out_sb[:, ot, :], in0=base[:, 0:1024],
                    scalar=proj_be[:, ot, :], in1=x_sb[:, ot, :],
                    op0=ALU.add, op1=ALU.add)
                q = nc.sync if ot % 2 == 0 else nc.scalar
                q.dma_start(out=out_r[:, ot, :], in_=out_sb[:, ot, :])

            def stage_b16(pv, g):
                """Denominator chain steps for pair pv, at slot g (0-15) of
                the following loop."""
                if g == 7:
                    recip_batch(pv, 0)
                    bc_load(pv, 0)
                    bc_load(pv, 1)
                elif g == 10:
                    pv_evac(pv, 0)
                elif g == 11:
                    pv_evac(pv, 1)
                elif g == 15:
                    recip_batch(pv, 1)
                    bc_load(pv, 2)
                    bc_load(pv, 3)

            def alloc_pT(t):
                pT_tiles[t] = pT_pool.tile([128, 2, MT, N], BF16, tag="pT", bufs=2,
                                           name=f"pT_{t}")

            # ---- pipeline emission ----
            # q0, k0 first (psSa/psSb) so pair-0 S^T can start early
            qk_tile(0, on_act=True, bases=[psSa])
            qk_tile(4, on_act=True, bases=[psSb])

            # pair 0: S^T+exp (all ACT; PE-bound anyway) + v tiles (psV) +
            # remaining qk tiles (psX)
            alloc_pT(0)
            rest = [1, 5, 2, 6, 3, 7]        # q1,k1,q2,k2,q3,k3
            for g in range(16):
                if g % 2 == 0:
                    v_tile(g // 2)
                elif g < 13:
                    qk_tile(rest[g // 2], on_act=False, bases=[psXa, psXb])
                st_half(0, g, on_dve=False)

            # pairs 1..3: PV(t-1) + S^T(t) + exp (ACT/DVE split) + stage_b
            for t in range(1, 4):
                alloc_pT(t)
                pv = t - 1
                for g in range(16):
                    if g == 0 and pv >= 1:
                        pv_evac(pv - 1, 2)
                    if g == 1 and pv >= 1:
                        pv_evac(pv - 1, 3)
                    if g % 2 == 0:
                        pv_chunk(pv, g // 4, (g // 2) % 2)
                    st_half(t, g, on_dve=(g in EXP_DVE))
                    stage_b16(pv, g)
                del pT_tiles[t - 1]

            # PV(3) + early proj partials (ot0/ot1 in psS halves, ot3 in psV)
            proj_sched = {1: (0, 0), 3: (1, 0), 5: (0, 1), 7: (1, 1),
                          9: (0, 2), 11: (1, 2)}
            for g in range(16):
                if g == 0:
                    pv_evac(2, 2)
                if g == 1:
                    pv_evac(2, 3)
                if g % 2 == 0:
                    pv_chunk(3, g // 4, (g // 2) % 2)
                ps = proj_sched.get(g)
                if ps is not None:
                    ot, kt = ps
                    proj_chunk(ot, [kt], psSa if ot == 0 else psSb,
                               first=(kt == 0), last=False)
                stage_b16(3, g)
            pv_evac(3, 2)
            pv_evac(3, 3)

            # ---- finish projection + residual ----
            proj_chunk(0, [3], psSa, first=False, last=True)
            proj_finish(0, psSa)
            proj_chunk(1, [3], psSb, first=False, last=True)
            proj_finish(1, psSb)
            proj_chunk(2, [0, 1, 2, 3], psSa, first=True, last=True)
            proj_finish(2, psSa)
            proj_chunk(3, [0, 1, 2, 3], psSb, first=True, last=True)
            proj_finish(3, psSb)

    nc.compile()
    return nc


def _host_prep(x, gn_w, gn_b, qkv_w, qkv_b, proj_w, proj_b):
    xf = np.ascontiguousarray(x.reshape(B, C, N), dtype=np.float32)
    import ml_dtypes
    qkv_wT = np.ascontiguousarray(qkv_w.T).astype(ml_dtypes.bfloat16)
    proj_wT = np.ascontiguousarray(proj_w.T).astype(ml_dtypes.bfloat16)
    proj_be = (proj_b + proj_w @ qkv_b[2 * C:]).astype(np.float32).reshape(C, 1)
    qk_bias = np.ascontiguousarray(qkv_b[:2 * C], dtype=np.float32).reshape(2 * C, 1)
    cid = np.arange(128)
    sel = ((cid[:, None] // GS == np.arange(8)[None, :]) / GS).astype(np.float32)
    expander = np.ascontiguousarray(
        (cid[:, None] // GS == np.arange(8)[None, :]).T.astype(np.float32))
    shared = {
        "qkv_wT": qkv_wT, "proj_wT": proj_wT, "qk_bias": qk_bias,
        "gn_w": np.asarray(gn_w, np.float32).reshape(C, 1),
        "gn_b": np.asarray(gn_b, np.float32).reshape(C, 1),
        "proj_be": proj_be, "sel": sel, "expander": expander,
    }
    return [{**shared, "x": np.ascontiguousarray(xf[i]),
             "x_bf": xf[i].astype(ml_dtypes.bfloat16)} for i in range(B)]


def kernel(x, gn_w, gn_b, qkv_w, qkv_b, proj_w, proj_b):
    from concourse import bass_utils
    in_maps = _host_prep(np.asarray(x), np.asarray(gn_w), np.asarray(gn_b),
                         np.asarray(qkv_w), np.asarray(qkv_b),
                         np.asarray(proj_w), np.asarray(proj_b))
    with_bias = bool(np.any(np.asarray(qkv_b)[:2 * C] != 0.0))
    key = ("nc", with_bias)
    if key not in _cache:
        _cache[key] = _build(with_bias)
    res = bass_utils.run_bass_kernel_spmd(_cache[key], in_maps,
                                          core_ids=list(range(B)), trace=TRACE)
    _cache["last_result"] = res
    out = np.stack([res.results[i]["out"] for i in range(B)])
    return out.reshape(B, C, 32, 32).astype(np.float32)
